# revision 1
# baseline (speedup 1.0000x reference)
"""Kimi-K2.5 tensorized MoE kernel for 8 TRN2 NeuronCores.

Sharding: expert-parallel. Core c owns routed experts [4c, 4c+4) and rows
[128c, 128(c+1)) of the shared-expert intermediate. The router runs
replicated on every core in fp32 (selection must match the reference
exactly). Expert/shared matmuls run in bf16 with fp32 PSUM accumulation.
Per-core partial outputs [H, T] are summed with a chunked ReduceScatter;
core c ends with rows [128c, 128(c+1)) of the summed transposed output.
The host concatenates the 8 shards and transposes back to [B, S, H].
"""

import sys

sys.path.insert(0, "/opt/trn_rl_repo")

import numpy as np
import ml_dtypes

from concourse import bass, bacc, mybir, tile
from concourse.bass_utils import run_bass_kernel_spmd

F32 = mybir.dt.float32
BF16 = mybir.dt.bfloat16
AF = mybir.ActivationFunctionType
ALU = mybir.AluOpType
AX = mybir.AxisListType

B, S, H = 2, 1024, 1024
T = B * S                 # 2048 tokens
I = 512                   # moe intermediate
E = 32                    # routed experts
TOP_K = 4
N_GROUP = 4
GRP = E // N_GROUP        # 8 experts per group
TOPK_GROUP = 2
SCALE = 2.5
SH_I = 1024               # shared intermediate (2 * I)
NCORES = 8
E_LOC = E // NCORES       # 4 experts per core
SH_LOC = SH_I // NCORES   # 128 shared-intermediate rows per core

USE_RS = True             # on-device ReduceScatter; False -> host-side sum

P = 128
TC = 512                  # t-chunk (moving free dim)
NT = T // TC              # 4 t-chunks
NTT = T // P              # 16 t-tiles of 128 tokens
NH = H // P               # 8 h-tiles
NI = I // P               # 4 i-tiles per expert


def _build(trace: bool = False):
    nc = bacc.Bacc("TRN2", target_bir_lowering=False, debug=False,
                   num_devices=NCORES)

    # ---- kernel I/O (per-core tensors; contents differ per core) ----
    tokf = nc.dram_tensor("tokf", [H, T], F32, kind="ExternalInput")
    tokb = nc.dram_tensor("tokb", [H, T], BF16, kind="ExternalInput")
    rwT = nc.dram_tensor("rwT", [H, E], F32, kind="ExternalInput")
    rbias = nc.dram_tensor("rbias", [1, E], F32, kind="ExternalInput")
    ident = nc.dram_tensor("ident", [P, P], F32, kind="ExternalInput")
    selb = nc.dram_tensor("selb", [E_LOC, E, P], F32, kind="ExternalInput")
    gwT = nc.dram_tensor("gwT", [E_LOC, H, I], BF16, kind="ExternalInput")
    uwT = nc.dram_tensor("uwT", [E_LOC, H, I], BF16, kind="ExternalInput")
    dwT = nc.dram_tensor("dwT", [E_LOC, I, H], BF16, kind="ExternalInput")
    sgwT = nc.dram_tensor("sgwT", [H, SH_LOC], BF16, kind="ExternalInput")
    suwT = nc.dram_tensor("suwT", [H, SH_LOC], BF16, kind="ExternalInput")
    sdwT = nc.dram_tensor("sdwT", [SH_LOC, H], BF16, kind="ExternalInput")
    if USE_RS:
        out_shard = nc.dram_tensor("out_shard", [P, T], F32,
                                   kind="ExternalOutput")
    else:
        out_shard = nc.dram_tensor("out_part", [H, T], F32,
                                   kind="ExternalOutput")

    rg = [list(range(NCORES))]

    with tile.TileContext(nc) as tc:
        with (
            tc.tile_pool(name="resident", bufs=1) as rp,
            tc.tile_pool(name="consts", bufs=1) as cp,
            tc.tile_pool(name="tokstream", bufs=2) as tp,
            tc.tile_pool(name="hid", bufs=1) as hp,
            tc.tile_pool(name="work", bufs=2) as xp,
            tc.tile_pool(name="router", bufs=2) as rr,
            tc.tile_pool(name="pmm", bufs=2, space="PSUM") as pmm,
            tc.tile_pool(name="pdown", bufs=1, space="PSUM") as pd,
            tc.tile_pool(name="pmisc", bufs=1, space="PSUM") as pm,
            tc.tile_pool(name="dram", bufs=1, space="DRAM") as dp,
        ):
            # ---------- constants ----------
            ones = cp.tile([1, P], F32, tag="ones")
            nc.vector.memset(ones[:], 1.0)
            ident_sb = cp.tile([P, P], F32, tag="ident")
            nc.sync.dma_start(ident_sb[:], ident[:, :])
            rbias_sb = cp.tile([1, E], F32, tag="rbias")
            nc.sync.dma_start(rbias_sb[:], rbias[:, :])
            selb_sb = []
            for el in range(E_LOC):
                t_ = cp.tile([E, P], F32, tag=f"selb{el}")
                nc.sync.dma_start(t_[:], selb[el, :, :])
                selb_sb.append(t_)
            rw_sb = []
            for ht in range(NH):
                t_ = cp.tile([P, E], F32, tag=f"rw{ht}")
                nc.sync.dma_start(t_[:], rwT[ht * P:(ht + 1) * P, :])
                rw_sb.append(t_)
            # bias broadcast [P, E] via ones^T @ rbias
            bias_ps = pm.tile([P, E], F32, tag="pwt")
            nc.tensor.matmul(bias_ps[:], ones[:], rbias_sb[:],
                             start=True, stop=True)
            bias_b = cp.tile([P, E], F32, tag="bias_b")
            nc.vector.tensor_copy(bias_b[:], bias_ps[:])

            # ---------- resident weights ----------
            dw_sb = {}
            for el in range(E_LOC):
                for it in range(NI):
                    t_ = rp.tile([P, H], BF16, tag=f"dw{el}_{it}")
                    nc.sync.dma_start(t_[:], dwT[el, it * P:(it + 1) * P, :])
                    dw_sb[(el, it)] = t_
            sgw_sb, suw_sb = [], []
            for ht in range(NH):
                t_ = rp.tile([P, SH_LOC], BF16, tag=f"sgw{ht}")
                nc.sync.dma_start(t_[:], sgwT[ht * P:(ht + 1) * P, :])
                sgw_sb.append(t_)
                t_ = rp.tile([P, SH_LOC], BF16, tag=f"suw{ht}")
                nc.sync.dma_start(t_[:], suwT[ht * P:(ht + 1) * P, :])
                suw_sb.append(t_)
            sdw_sb = rp.tile([SH_LOC, H], BF16, tag="sdw")
            nc.sync.dma_start(sdw_sb[:], sdwT[:, :])
            gw_sb, uw_sb = {}, {}
            for el in range(E_LOC):
                for ht in range(NH):
                    t_ = rp.tile([P, I], BF16, tag=f"gw{el}_{ht}")
                    nc.sync.dma_start(t_[:], gwT[el, ht * P:(ht + 1) * P, :])
                    gw_sb[(el, ht)] = t_
                    t_ = rp.tile([P, I], BF16, tag=f"uw{el}_{ht}")
                    nc.sync.dma_start(t_[:], uwT[el, ht * P:(ht + 1) * P, :])
                    uw_sb[(el, ht)] = t_

            # ---------- router (fp32, replicated) ----------
            # one PSUM accumulation group per t-tile (interleaved groups in
            # one bank clobber each other: start clears the whole bank).
            scores_sb = cp.tile([P, NTT * E], F32, tag="scores")
            for tt in range(NTT):
                lgp = pm.tile([P, E], F32, tag="plg")
                for ht in range(NH):
                    tfm = rr.tile([P, P], F32, tag="tokf")
                    nc.sync.dma_start(
                        tfm[:], tokf[ht * P:(ht + 1) * P, tt * P:(tt + 1) * P])
                    nc.tensor.matmul(lgp[:], tfm[:], rw_sb[ht][:],
                                     start=(ht == 0), stop=(ht == NH - 1))
                nc.scalar.activation(scores_sb[:, tt * E:(tt + 1) * E],
                                     lgp[:], AF.Sigmoid)

            # per t-tile top-k group logic -> W^T stored as wt_sb [E, T]
            wt_sb = cp.tile([E, T], F32, tag="wt")
            for tt in range(NTT):
                sc = scores_sb[:, tt * E:(tt + 1) * E]
                sfc = rr.tile([P, E], F32, tag="sfc")
                nc.vector.tensor_add(sfc[:], sc, bias_b[:])
                m1 = rr.tile([P, N_GROUP], F32, tag="m1")
                eq = rr.tile([P, E], F32, tag="eq")
                for g in range(N_GROUP):
                    gsl = slice(g * GRP, (g + 1) * GRP)
                    nc.vector.tensor_reduce(m1[:, g:g + 1], sfc[:, gsl],
                                            axis=AX.X, op=ALU.max)
                    nc.vector.tensor_scalar(eq[:, gsl], sfc[:, gsl],
                                            m1[:, g:g + 1], None,
                                            op0=ALU.is_equal)
                sfc_wo = rr.tile([P, E], F32, tag="sfc_wo")
                tmp32 = rr.tile([P, E], F32, tag="tmp32")
                nc.vector.tensor_scalar_mul(tmp32[:], eq[:], 1e30)
                nc.vector.tensor_sub(sfc_wo[:], sfc[:], tmp32[:])
                m2 = rr.tile([P, N_GROUP], F32, tag="m2")
                for g in range(N_GROUP):
                    gsl = slice(g * GRP, (g + 1) * GRP)
                    nc.vector.tensor_reduce(m2[:, g:g + 1], sfc_wo[:, gsl],
                                            axis=AX.X, op=ALU.max)
                gs = rr.tile([P, N_GROUP], F32, tag="gs")
                nc.vector.tensor_add(gs[:], m1[:], m2[:])
                gm1 = rr.tile([P, 1], F32, tag="gm1")
                nc.vector.tensor_reduce(gm1[:], gs[:], axis=AX.X, op=ALU.max)
                eqg = rr.tile([P, N_GROUP], F32, tag="eqg")
                nc.vector.tensor_scalar(eqg[:], gs[:], gm1[:], None,
                                        op0=ALU.is_equal)
                gs2 = rr.tile([P, N_GROUP], F32, tag="gs2")
                tmp4 = rr.tile([P, N_GROUP], F32, tag="tmp4")
                nc.vector.tensor_scalar_mul(tmp4[:], eqg[:], 1e30)
                nc.vector.tensor_sub(gs2[:], gs[:], tmp4[:])
                gm2 = rr.tile([P, 1], F32, tag="gm2")
                nc.vector.tensor_reduce(gm2[:], gs2[:], axis=AX.X, op=ALU.max)
                gmask = rr.tile([P, N_GROUP], F32, tag="gmask")
                nc.vector.tensor_scalar(gmask[:], gs[:], gm2[:], None,
                                        op0=ALU.is_ge)
                masked = rr.tile([P, E], F32, tag="masked")
                for g in range(N_GROUP):
                    gsl = slice(g * GRP, (g + 1) * GRP)
                    nc.vector.tensor_scalar(masked[:, gsl], sfc[:, gsl],
                                            gmask[:, g:g + 1], None,
                                            op0=ALU.mult)
                sel = rr.tile([P, E], F32, tag="sel")
                nc.vector.memset(sel[:], 0.0)
                for _k in range(TOP_K):
                    mk = rr.tile([P, 1], F32, tag="mk")
                    nc.vector.tensor_reduce(mk[:], masked[:], axis=AX.X,
                                            op=ALU.max)
                    eqk = rr.tile([P, E], F32, tag="eqk")
                    nc.vector.tensor_scalar(eqk[:], masked[:], mk[:], None,
                                            op0=ALU.is_equal)
                    nc.vector.tensor_add(sel[:], sel[:], eqk[:])
                    prod = rr.tile([P, E], F32, tag="prod")
                    nc.vector.tensor_mul(prod[:], masked[:], eqk[:])
                    nc.vector.tensor_sub(masked[:], masked[:], prod[:])
                wun = rr.tile([P, E], F32, tag="wun")
                nc.vector.tensor_tensor(wun[:], sc, sel[:], op=ALU.mult)
                den = rr.tile([P, 1], F32, tag="den")
                nc.vector.tensor_reduce(den[:], wun[:], axis=AX.X, op=ALU.add)
                nc.vector.tensor_scalar_add(den[:], den[:], 1e-20)
                rec = rr.tile([P, 1], F32, tag="rec")
                nc.vector.reciprocal(rec[:], den[:])
                nc.vector.tensor_scalar_mul(rec[:], rec[:], SCALE)
                wfin = rr.tile([P, E], F32, tag="wfin")
                nc.vector.tensor_scalar_mul(wfin[:], wun[:], rec[:])
                # transpose [P, E] -> [E, P] and stash in wt_sb
                wt_ps = pm.tile([E, P], F32, tag="pwt")
                nc.tensor.transpose(wt_ps[:], wfin[:], ident_sb[:])
                nc.vector.tensor_copy(wt_sb[:, tt * P:(tt + 1) * P], wt_ps[:])

            # ---------- expert phase ----------
            for tcx in range(NT):
                tsl = slice(tcx * TC, (tcx + 1) * TC)
                tokc = []
                for ht in range(NH):
                    t_ = tp.tile([P, TC], BF16, tag=f"tokc{ht}")
                    nc.sync.dma_start(t_[:], tokb[ht * P:(ht + 1) * P, tsl])
                    tokc.append(t_)
                # shared expert hidden (no routing weight)
                sg_ps = pmm.tile([P, TC], F32, tag="g_ps")
                su_ps = pmm.tile([P, TC], F32, tag="u_ps")
                for ht in range(NH):
                    nc.tensor.matmul(sg_ps[:], sgw_sb[ht][:], tokc[ht][:],
                                     start=(ht == 0), stop=(ht == NH - 1))
                for ht in range(NH):
                    nc.tensor.matmul(su_ps[:], suw_sb[ht][:], tokc[ht][:],
                                     start=(ht == 0), stop=(ht == NH - 1))
                sh_act = xp.tile([P, TC], F32, tag="gact")
                nc.scalar.activation(sh_act[:], sg_ps[:], AF.Silu)
                sh_hid = hp.tile([P, TC], BF16, tag="sh_hid")
                nc.vector.tensor_tensor(sh_hid[:], sh_act[:], su_ps[:],
                                        op=ALU.mult)

                hid = {}
                for el in range(E_LOC):
                    # routing weights for this (expert, t-chunk), broadcast
                    # across partitions: selb[el].T @ wt  -> [P, TC]
                    wb_ps = pm.tile([P, TC], F32, tag="pwb")
                    nc.tensor.matmul(wb_ps[:], selb_sb[el][:], wt_sb[:, tsl],
                                     start=True, stop=True)
                    wb_sb = xp.tile([P, TC], F32, tag="wb")
                    nc.scalar.copy(wb_sb[:], wb_ps[:])
                    for it in range(NI):
                        isl = slice(it * P, (it + 1) * P)
                        g_ps = pmm.tile([P, TC], F32, tag="g_ps")
                        u_ps = pmm.tile([P, TC], F32, tag="u_ps")
                        for ht in range(NH):
                            nc.tensor.matmul(g_ps[:], gw_sb[(el, ht)][:, isl],
                                             tokc[ht][:],
                                             start=(ht == 0),
                                             stop=(ht == NH - 1))
                        for ht in range(NH):
                            nc.tensor.matmul(u_ps[:], uw_sb[(el, ht)][:, isl],
                                             tokc[ht][:],
                                             start=(ht == 0),
                                             stop=(ht == NH - 1))
                        gact = xp.tile([P, TC], F32, tag="gact")
                        nc.scalar.activation(gact[:], g_ps[:], AF.Silu)
                        uwv = xp.tile([P, TC], F32, tag="uwv")
                        nc.vector.tensor_tensor(uwv[:], u_ps[:], wb_sb[:],
                                                op=ALU.mult)
                        ht_ = hp.tile([P, TC], BF16, tag=f"hid{el}_{it}")
                        nc.vector.tensor_tensor(ht_[:], gact[:], uwv[:],
                                                op=ALU.mult)
                        hid[(el, it)] = ht_

                if USE_RS:
                    cc_in = dp.tile([H, TC], F32, tag=f"cc_in{tcx}")
                else:
                    cc_in = None
                for ht in range(NH):
                    hsl = slice(ht * P, (ht + 1) * P)
                    d_ps = pd.tile([P, TC], F32, tag="d_ps")
                    k = 0
                    for el in range(E_LOC):
                        for it in range(NI):
                            nc.tensor.matmul(d_ps[:],
                                             dw_sb[(el, it)][:, hsl],
                                             hid[(el, it)][:],
                                             start=(k == 0), stop=False)
                            k += 1
                    nc.tensor.matmul(d_ps[:], sdw_sb[:, hsl], sh_hid[:],
                                     start=False, stop=True)
                    o_sb = xp.tile([P, TC], F32, tag="o_sb")
                    nc.scalar.copy(o_sb[:], d_ps[:])
                    if USE_RS:
                        nc.sync.dma_start(cc_in[hsl, :], o_sb[:])
                    else:
                        nc.sync.dma_start(out_shard[hsl, tsl], o_sb[:])

                if USE_RS:
                    cc_out = dp.tile([P, TC], F32, tag=f"cc_out{tcx}")
                    nc.gpsimd.collective_compute(
                        "ReduceScatter", ALU.add, replica_groups=rg,
                        ins=[cc_in.opt()], outs=[cc_out.opt()],
                    )
                    nc.gpsimd.dma_start(out_shard[:, tsl], cc_out[:])

    nc.compile()
    return nc


def _prep_inputs(hidden_states, router_weight, router_bias, gate_w, up_w,
                 down_w, shared_gate_w, shared_up_w, shared_down_w):
    bf = ml_dtypes.bfloat16
    tokens = np.ascontiguousarray(
        np.asarray(hidden_states, dtype=np.float32).reshape(T, H))
    tokf = np.ascontiguousarray(tokens.T)                       # [H, T] f32
    tokb = tokf.astype(bf)
    rwT = np.ascontiguousarray(
        np.asarray(router_weight, dtype=np.float32).T)          # [H, E]
    rbias = np.asarray(router_bias, dtype=np.float32).reshape(1, E)
    ident = np.eye(P, dtype=np.float32)
    gwT = np.ascontiguousarray(
        np.asarray(gate_w, dtype=np.float32).transpose(0, 2, 1)).astype(bf)
    uwT = np.ascontiguousarray(
        np.asarray(up_w, dtype=np.float32).transpose(0, 2, 1)).astype(bf)
    dwT = np.ascontiguousarray(
        np.asarray(down_w, dtype=np.float32).transpose(0, 2, 1)).astype(bf)
    sgwT = np.ascontiguousarray(
        np.asarray(shared_gate_w, dtype=np.float32).T)          # [H, SH_I]
    suwT = np.ascontiguousarray(
        np.asarray(shared_up_w, dtype=np.float32).T)
    sdwT = np.ascontiguousarray(
        np.asarray(shared_down_w, dtype=np.float32).T)          # [SH_I, H]

    in_maps = []
    for c in range(NCORES):
        esl = slice(c * E_LOC, (c + 1) * E_LOC)
        ssl = slice(c * SH_LOC, (c + 1) * SH_LOC)
        sel = np.zeros((E_LOC, E, P), dtype=np.float32)
        for el in range(E_LOC):
            sel[el, c * E_LOC + el, :] = 1.0
        in_maps.append({
            "tokf": tokf,
            "tokb": tokb,
            "rwT": rwT,
            "rbias": rbias,
            "ident": ident,
            "selb": sel,
            "gwT": np.ascontiguousarray(gwT[esl]),
            "uwT": np.ascontiguousarray(uwT[esl]),
            "dwT": np.ascontiguousarray(dwT[esl]),
            "sgwT": np.ascontiguousarray(sgwT[:, ssl]).astype(bf),
            "suwT": np.ascontiguousarray(suwT[:, ssl]).astype(bf),
            "sdwT": np.ascontiguousarray(sdwT[ssl, :]).astype(bf),
        })
    return in_maps


def run_on_device(inputs: dict, trace: bool = False):
    in_maps = _prep_inputs(**inputs)
    nc = _build(trace=trace)
    res = run_bass_kernel_spmd(nc, in_maps, list(range(NCORES)), trace=trace)
    if USE_RS:
        shards = [res.results[c]["out_shard"] for c in range(NCORES)]
        outT = np.concatenate(shards, axis=0)                   # [H, T]
    else:
        outT = np.sum([res.results[c]["out_part"] for c in range(NCORES)],
                      axis=0, dtype=np.float32)
    out = np.ascontiguousarray(outT.T).reshape(B, S, H).astype(np.float32)
    return out, res


def kernel(**inputs) -> np.ndarray:
    out, _ = run_on_device(inputs, trace=False)
    return out


def bench(inputs: dict, iters: int = 5):
    """Compile once, execute repeatedly, report wall time per call."""
    import time
    import jax
    from jax.sharding import Mesh, PartitionSpec
    from jax.experimental.shard_map import shard_map
    from concourse import bass2jax, mybir as mb

    in_maps = _prep_inputs(**inputs)
    nc = _build()
    bass2jax.install_neuronx_cc_hook()
    partition_name = (nc.partition_id_tensor.name
                      if nc.partition_id_tensor else None)
    in_names, out_names, out_avals, zero_outs = [], [], [], []
    for alloc in nc.m.functions[0].allocations:
        if not isinstance(alloc, mb.MemoryLocationSet):
            continue
        name = alloc.memorylocations[0].name
        if alloc.kind == "ExternalInput":
            if name != partition_name:
                in_names.append(name)
        elif alloc.kind == "ExternalOutput":
            shape = tuple(alloc.tensor_shape)
            dtype = mb.dt.np(alloc.dtype)
            out_names.append(name)
            out_avals.append(jax.core.ShapedArray(shape, dtype))
            zero_outs.append(np.zeros(shape, dtype))
    n_params = len(in_names)
    n_outs = len(out_avals)
    in_names_all = in_names + out_names + ([partition_name]
                                           if partition_name else [])

    def _body(*args):
        operands = list(args)
        if partition_name is not None:
            operands.append(bass2jax.partition_id_tensor())
        return tuple(bass2jax._bass_exec_p.bind(
            *operands, out_avals=tuple(out_avals),
            in_names=tuple(in_names_all), out_names=tuple(out_names),
            lowering_input_output_aliases=(),
            sim_require_finite=True, sim_require_nnan=True, nc=nc))

    devices = jax.devices()[:NCORES]
    mesh = Mesh(np.asarray(devices), ("core",))
    sharded = jax.jit(
        shard_map(_body, mesh=mesh,
                  in_specs=(PartitionSpec("core"),) * (n_params + n_outs),
                  out_specs=(PartitionSpec("core"),) * n_outs,
                  check_rep=False),
        keep_unused=True)
    concat_in = [np.concatenate([np.asarray(in_maps[c][n])
                                 for c in range(NCORES)], axis=0)
                 for n in in_names]
    concat_zero = [np.zeros((NCORES * z.shape[0], *z.shape[1:]), z.dtype)
                   for z in zero_outs]
    sharding = jax.sharding.NamedSharding(mesh, PartitionSpec("core"))
    dev_in = [jax.device_put(a, sharding) for a in concat_in]
    dev_zero = [jax.device_put(a, sharding) for a in concat_zero]
    times = []
    for _ in range(iters):
        t0 = time.perf_counter()
        outs = sharded(*dev_in, *dev_zero)
        jax.block_until_ready(outs)
        times.append(time.perf_counter() - t0)
    return times, outs, out_names

